# revision 2
# baseline (speedup 1.0000x reference)
"""Trainium2 Bass kernel v5 for PixelUnshuffle->MHA->PixelShuffle.

v2 -> v3 (v2 = 239us):
  - ONE PSUM pool for phases B/C/D (pool boundaries in v2 serialized the
    phases behind the softmax-normalization chain).
  - Softmax 1/Z via ACT Ln -> K=1 f32 broadcast matmul -> ACT Exp(scale=-1)
    (v2's InstReciprocal on a single partition cost 6.5us per head and
    stalled the in-order PE queue at every head-pair seam).
  - PE-queue-aware emission order: qk(heads 2,3) groups are emitted BEFORE
    the hp0 norm chain, and the first out-projection ic0 matmuls BEFORE the
    hp1 norm chain, so the PE always has independent work queued ahead of
    any matmul that waits on a slow cross-engine chain.
  - WqC/Wo input DMAs deferred out of phase A's DMA window (A is DMA-bound).
  - Finer first-chunk DMAs + memset/warmup as the very first instructions.

Phases:
  A: qk(heads 0,1) + v(m 0..3)     [own 8-bank pool, DMA-paced]
  B: attention heads 0,1; fillers: v(m 4..7), then qk(heads 2,3) j=0..2
  C: attention heads 2,3; fillers: qk j=3, then out-proj ic0 of cb 0..2
  D: output projection, evac alternating Scalar/Vector, batched out-DMA

Layouts: see docstring of kernel_v2 (unchanged).
"""

import sys

if "/opt/trn_rl_repo" not in sys.path:
    sys.path.insert(0, "/opt/trn_rl_repo")

import os

import ml_dtypes
import numpy as np

import concourse.bass as bass
from concourse import bacc, mybir, tile
from concourse.bass_utils import run_bass_kernel_spmd

F32 = mybir.dt.float32
BF16 = mybir.dt.bfloat16
EXP = mybir.ActivationFunctionType.Exp
LN = mybir.ActivationFunctionType.Ln

SCALE = 0.125  # DIM_HEAD ** -0.5

_CACHE = {}


def _build(zero_bias=True, debug_outs=False):
    nc = bacc.Bacc("TRN2", target_bir_lowering=False, debug=False, num_devices=8)

    tt_d = nc.dram_tensor("Tt", [32, 128, 1024], BF16, kind="ExternalInput").ap()
    wqa_d = nc.dram_tensor("WqA", [32, 128, 512], BF16, kind="ExternalInput").ap()
    wqc_d = nc.dram_tensor("WqC", [32, 128, 256], BF16, kind="ExternalInput").ap()
    wo_d = nc.dram_tensor("Wo", [256, 4096], BF16, kind="ExternalInput").ap()
    b_d = nc.dram_tensor("bias", [128, 32], F32, kind="ExternalInput").ap()
    out_d = nc.dram_tensor("out", [32, 128, 1024], BF16, kind="ExternalOutput").ap()

    dbg = None
    if debug_outs:
        dbg = {
            "qkT": nc.dram_tensor("dbg_qkT", [128, 4, 1024], BF16, kind="ExternalOutput").ap(),
            "v_sb": nc.dram_tensor("dbg_v", [128, 8, 4, 68], BF16, kind="ExternalOutput").ap(),
            "outT": nc.dram_tensor("dbg_outT", [128, 2, 1024], BF16, kind="ExternalOutput").ap(),
        }

    def dram_ap(base, off, pattern):
        return bass.AP(tensor=base.tensor, offset=base.offset + off, ap=pattern)

    with tile.TileContext(nc) as tc:
        _build_tiled(nc, tc, tt_d, wqa_d, wqc_d, wo_d, b_d, out_d, dram_ap,
                     zero_bias, dbg)
    nc.compile()
    return nc


def _build_tiled(nc, tc, tt_d, wqa_d, wqc_d, wo_d, b_d, out_d, dram_ap,
                 zero_bias, dbg=None):
    from contextlib import ExitStack

    def copy_sc(dst, src):
        nc.scalar.copy(dst, src)

    def copy_ve(dst, src):
        nc.vector.tensor_copy(dst, src)

    with ExitStack() as ctx:
        pers = ctx.enter_context(tc.tile_pool(name="pers", bufs=1))

        # ---- persistent SBUF tiles ----
        tt_sb = pers.tile([128, 32, 1024], BF16)   # 64KB/part
        wqa_sb = pers.tile([128, 32, 512], BF16)   # 32KB
        wqc_sb = pers.tile([128, 32, 256], BF16)   # 16KB
        wo_sb = pers.tile([128, 2, 4096], BF16)    # 16KB
        qkT = pers.tile([128, 4, 1024], BF16)      # 8KB
        v_sb = pers.tile([128, 8, 4, 68], BF16)    # 4.25KB
        ed = pers.tile([128, 2, 8, 1024], BF16)    # 32KB (h2, mc, n)
        outT = pers.tile([128, 2, 1024], BF16)     # 4KB
        onrm = pers.tile([64, 1024], BF16)         # h2=1 staging (shared)
        zraw = pers.tile([1, 1024], F32)           # Z row staged to SBUF
        zrec = pers.tile([1, 1024], F32)           # 1/Z, reused per head
        zbc_sb = pers.tile([64, 1024], BF16)       # 1/Z broadcast, reused
        ones64f = pers.tile([1, 64], F32)
        bias_sb = pers.tile([128, 32], F32)
        warm = pers.tile([128, 512], BF16)
        expd = pers.tile([1, 16], BF16)

        # ---- init: DVE memset first so PE warmup can start immediately ----
        nc.vector.memset(warm[:], 0.0)
        nc.vector.memset(ones64f[:], 1.0)
        nc.vector.memset(v_sb[:, :, :, 64:68], 1.0)
        # preload the ln/exp table set (~2.7us) long before it's needed
        nc.scalar.activation(expd[:], warm[0:1, 0:16], EXP)

        # ---- phase-A input DMAs (A is DMA-bound) ----
        # 4-queue rotation; chunk-size ramp: singles first (fast arrival),
        # then pairs, then quads (amortize per-DMA DGE overhead)
        qrot = [nc.gpsimd, nc.scalar, nc.sync, nc.gpsimd]
        qi = 0

        def in_dma(dst, src_off, base, nchunk, width):
            nonlocal_qi = qrot[in_dma.qi % 4]
            in_dma.qi += 1
            nonlocal_qi.dma_start(
                out=dst,
                in_=dram_ap(base, src_off,
                            [[width, 128], [width * 128, nchunk], [1, width]]
                            if nchunk > 1 else [[width, 128], [1, width]]),
            )
        in_dma.qi = 0

        ramp = ([(ci, 1) for ci in range(4)]
                + [(ci, 2) for ci in range(4, 20, 2)]
                + [(ci, 4) for ci in range(20, 32, 4)])
        for ci, n in ramp:
            in_dma(tt_sb[:, ci : ci + n, :], ci * 131072, tt_d, n, 1024)
            in_dma(wqa_sb[:, ci : ci + n, :], ci * 65536, wqa_d, n, 512)
        # later-phase weights at the tail of the same queues: they issue
        # after every phase-A chunk and stream during early B
        for j in range(4):
            in_dma(wqc_sb[:, 8 * j : 8 * j + 8, :], j * 8 * 32768, wqc_d,
                   8, 256)
        for j in range(2):
            qrot[j].dma_start(
                out=wo_sb[:, :, j * 2048 : (j + 1) * 2048],
                in_=dram_ap(wo_d, j * 2048,
                            [[4096, 128], [524288, 2], [1, 2048]]),
            )
        nc.scalar.dma_start(out=bias_sb[:], in_=dram_ap(b_d, 0, [[32, 128], [1, 32]]))

        # ================= phase A: qk(hp0) + v(m 0..3) =================
        with tc.tile_pool(name="psA", bufs=1, space="PSUM") as psA:
            qk_ps = [
                psA.tile([128, 512], F32, tag="qk", bufs=4, name=f"qkps_{j}")
                for j in range(4)  # j = (ot0,w0) (ot0,w1) (ot2,w0) (ot2,w1)
            ]
            v_ps = [
                psA.tile([128, 256], F32, tag="vv", bufs=4, name=f"vps_{mc}")
                for mc in range(4)
            ]
            # warmup into qk_ps[0]'s bank (real start=True clears it)
            for _ in range(14):
                nc.tensor.matmul(qk_ps[0][:], warm[:, 0:128], warm[:],
                                 start=True, stop=True)
            for ci in range(32):
                first, last = ci == 0, ci == 31
                for j in range(4):
                    ot, w = divmod(j, 2)
                    nc.tensor.matmul(
                        qk_ps[j][:],
                        wqa_sb[:, ci, ot * 128 : (ot + 1) * 128],
                        tt_sb[:, ci, w * 512 : (w + 1) * 512],
                        start=first, stop=last,
                    )
                    nc.tensor.matmul(
                        v_ps[j][:],
                        tt_sb[:, ci, j * 128 : (j + 1) * 128],
                        wqa_sb[:, ci, 256:512],
                        start=first, stop=last,
                    )
            for j in range(4):
                ot, w = divmod(j, 2)
                cp = copy_sc if j % 2 == 0 else copy_ve
                cp(qkT[:, 2 * ot, w * 512 : (w + 1) * 512], qk_ps[j][:])
            for mc in range(4):
                copy_ve(v_sb[:, mc, :, 0:64],
                        v_ps[mc][:].rearrange("p (h d) -> p h d", h=4))

        # ============ phases B, C, D share ONE pool (no barriers) ============
        # tags: rot  3x [128,512] f32  (1 bank each)   - dts / zb / y ring
        #       vrot 1x [128,512]      (1 bank)        - v / qk / zb / y ring
        #       oaug 2x [68,2,512]     (2 banks each)  - oaug pairs / y ring
        with tc.tile_pool(name="psBCD", bufs=1, space="PSUM") as ps:
            def dots_mc(hp, mc):
                for nh in range(2):
                    pair = []
                    for h2 in range(2):
                        dt = ps.tile([128, 512], F32, tag="rot", bufs=3,
                                     name=f"dt{hp}_{mc}_{nh}_{h2}")
                        b = h2 * 64
                        nc.tensor.matmul(
                            dt[:],
                            qkT[b : b + 64, 2 + hp, mc * 128 : (mc + 1) * 128],
                            qkT[b : b + 64, hp, nh * 512 : (nh + 1) * 512],
                            start=True, stop=True,
                        )
                        pair.append(dt)
                    for h2 in range(2):
                        nc.scalar.activation(
                            ed[:, h2, mc, nh * 512 : (nh + 1) * 512],
                            pair[h2][:], EXP, scale=SCALE,
                        )

            def oaug_mc(oaug, hp, mc):
                for h2 in range(2):
                    for nh in range(2):
                        nc.tensor.matmul(
                            oaug[h2][0:68, nh, :],
                            v_sb[:, mc, 2 * hp + h2, :],
                            ed[:, h2, mc, nh * 512 : (nh + 1) * 512],
                            start=mc == 0, stop=mc == 7,
                        )

            def norm_hp(oaug, hp, ztag, zbufs):
                """outT[:, hp] = oaug * (1/Z). DVE fast-reciprocal on the
                PSUM Z row; a K=1 f32 matmul broadcasts 1/Z to 64 partitions
                (PE work that fills the seam); ACT stages it to SBUF."""
                for h2 in range(2):
                    h = 2 * hp + h2
                    nc.scalar.copy(
                        zraw[0:1, :],
                        oaug[h2][64:65, :, :].rearrange("p a b -> p (a b)"),
                    )
                    nc.vector.reciprocal_approx_fast(
                        zrec[0:1, :], zraw[0:1, :])
                    for nh in range(2):
                        zb = ps.tile([64, 512], F32, tag=ztag, bufs=zbufs,
                                     name=f"zb{hp}_{h2}_{nh}")
                        nc.tensor.matmul(
                            zb[:], ones64f[0:1, :],
                            zrec[0:1, nh * 512 : (nh + 1) * 512],
                            start=True, stop=True,
                        )
                        nc.scalar.copy(
                            zbc_sb[:, nh * 512 : (nh + 1) * 512], zb[:],
                        )
                        dst = (outT[0:64, hp, nh * 512 : (nh + 1) * 512]
                               if h2 == 0 else
                               onrm[0:64, nh * 512 : (nh + 1) * 512])
                        nc.vector.tensor_mul(
                            dst,
                            oaug[h2][0:64, nh, :],
                            zbc_sb[:, nh * 512 : (nh + 1) * 512],
                        )
                nc.sync.dma_start(out=outT[64:128, hp, :], in_=onrm[:, :])

            # ---------------- phase B ----------------
            oaugB = [
                ps.tile([68, 2, 512], F32, tag="oaug", bufs=2, name=f"oaugB{h2}")
                for h2 in range(2)
            ]
            dots_mc(0, 0)
            dots_mc(0, 1)
            vg = None
            for mc in range(8):
                g = 4 + mc // 2
                if mc % 2 == 0:
                    vg = ps.tile([128, 256], F32, tag="vrot", bufs=1,
                                 name=f"vpsB_{g}")
                lo, hi = (0, 16) if mc % 2 == 0 else (16, 32)
                for ci in range(lo, hi):
                    nc.tensor.matmul(
                        vg[:],
                        tt_sb[:, ci, g * 128 : (g + 1) * 128],
                        wqa_sb[:, ci, 256:512],
                        start=ci == 0, stop=ci == 31,
                    )
                if mc % 2 == 1:
                    copy_ve(v_sb[:, g, :, 0:64],
                            vg[:].rearrange("p (h d) -> p h d", h=4))
                oaug_mc(oaugB, 0, mc)
                if mc < 6:
                    dots_mc(0, mc + 2)

            # B tail: qk(hp1) groups j=0..2 BEFORE the hp0 norm chain, so the
            # PE has ~10us of queued work while Ln/Exp/muls run cross-engine
            def qk_group_c(j, lo, hi, qp):
                for ci in range(lo, hi):
                    nc.tensor.matmul(
                        qp[:],
                        wqc_sb[:, ci, (j // 2) * 128 : (j // 2 + 1) * 128],
                        tt_sb[:, ci, (j % 2) * 512 : (j % 2 + 1) * 512],
                        start=ci == 0, stop=ci == 31,
                    )

            for j in range(2):
                qp = ps.tile([128, 512], F32, tag="vrot", bufs=1, name=f"qkC{j}")
                qk_group_c(j, 0, 32, qp)
                ot, w = 1 + 2 * (j // 2), j % 2
                copy_ve(qkT[:, ot, w * 512 : (w + 1) * 512], qp[:])
            norm_hp(oaugB, 0, "rot", 3)

            # ---------------- phase C ----------------
            oaugC = [
                ps.tile([68, 2, 512], F32, tag="oaug", bufs=2, name=f"oaugC{h2}")
                for h2 in range(2)
            ]
            # k(ot3) w0 full group first: dots(0..3) read it
            qp2 = ps.tile([128, 512], F32, tag="vrot", bufs=1, name="qkC2")
            qk_group_c(2, 0, 32, qp2)
            copy_ve(qkT[:, 3, 0:512], qp2[:])
            dots_mc(1, 0)
            dots_mc(1, 1)
            qp3 = None
            j3_bursts = [(0, 11), (11, 22), (22, 32)]
            for mc in range(8):
                if mc < 3:  # k(ot3) w1 filler; its evac MUST be emitted
                    # before dots(4)'s emission (same-iteration, later) or
                    # the read binds to no writer
                    if mc == 0:
                        qp3 = ps.tile([128, 512], F32, tag="vrot", bufs=1,
                                      name="qkC3")
                    lo, hi = j3_bursts[mc]
                    qk_group_c(3, lo, hi, qp3)
                    if mc == 2:
                        copy_ve(qkT[:, 3, 512:1024], qp3[:])
                oaug_mc(oaugC, 1, mc)
                if mc < 6:
                    dots_mc(1, mc + 2)

            # seam fill: ic0 of out-projection for cb 0..2 only needs
            # outT[:, 0] (done since B) - queue it ahead of the hp1 norm
            def ymm(yp, cb, half, ic, start, stop):
                nc.tensor.matmul(
                    yp[:],
                    wo_sb[:, ic, cb * 128 : (cb + 1) * 128],
                    outT[:, ic, half * 512 : (half + 1) * 512],
                    start=start, stop=stop,
                )

            # exactly 3 rot slots exist: prefill exactly 3 open ic0
            # accumulations (a 4th would block the PE queue ahead of the
            # norm chain that frees the slots -> deadlock)
            pending = {}
            for cb, half in [(0, 0), (0, 1), (1, 0)]:
                yp = ps.tile([128, 512], F32, tag="rot", bufs=3,
                             name=f"y{cb}_{half}")
                ymm(yp, cb, half, 0, True, False)
                pending[(cb, half)] = yp

            norm_hp(oaugC, 1, "vrot", 1)

            if dbg is not None:
                nc.gpsimd.dma_start(out=dbg["qkT"][:], in_=qkT[:])
                nc.gpsimd.dma_start(out=dbg["v_sb"][:], in_=v_sb[:])
                nc.gpsimd.dma_start(out=dbg["outT"][:], in_=outT[:])

            # ---------------- phase D ----------------
            def yevac(yp, ybf, cb, half):
                dst = ybf[:, half * 512 : (half + 1) * 512]
                if zero_bias:
                    cp = copy_sc if (cb + half) % 2 == 0 else copy_ve
                    cp(dst, yp[:])
                else:
                    nc.vector.tensor_scalar_add(dst, yp[:], bias_sb[:, cb : cb + 1])

            def yout(cb, ybf):
                if cb >= 28:  # split the tail blocks across both queues
                    for hh in range(2):
                        eng = nc.sync if hh == 0 else nc.gpsimd
                        eng.dma_start(
                            out=dram_ap(out_d, cb * 131072 + hh * 512,
                                        [[1024, 128], [1, 512]]),
                            in_=ybf[:, hh * 512 : (hh + 1) * 512],
                        )
                    return
                eng = nc.sync if cb % 2 == 0 else nc.gpsimd
                eng.dma_start(
                    out=dram_ap(out_d, cb * 131072, [[1024, 128], [1, 1024]]),
                    in_=ybf[:],
                )

            # remaining blocks: y tiles round-robin across all three tags
            tags = [("rot", 3), ("rot", 3), ("rot", 3), ("oaug", 2),
                    ("oaug", 2)]
            for cb in range(32):
                ybf = pers.tile([128, 1024], BF16, tag="ybf", bufs=5,
                                name=f"ybf{cb}")
                for half in range(2):
                    yp = pending.pop((cb, half), None)
                    if yp is None:
                        tg, tb = tags[(2 * cb + half) % 5]
                        yp = ps.tile([128, 512], F32, tag=tg, bufs=tb,
                                     name=f"y{cb}_{half}")
                        ymm(yp, cb, half, 0, True, False)
                    ymm(yp, cb, half, 1, False, True)
                    yevac(yp, ybf, cb, half)
                yout(cb, ybf)


def _get_nc(zero_bias, debug_outs=False):
    key = f"nc_v5_zb{int(zero_bias)}_dbg{int(debug_outs)}"
    if key not in _CACHE:
        _CACHE[key] = _build(zero_bias=zero_bias, debug_outs=debug_outs)
    return _CACHE[key]


def _prep_weights(W_qkv, W_out, b_out):
    bf = ml_dtypes.bfloat16
    wqa = np.ascontiguousarray(
        np.concatenate([W_qkv[:, 0:128], W_qkv[:, 256:384], W_qkv[:, 512:768]], 1)
    ).astype(bf).reshape(32, 128, 512)
    wqc = np.ascontiguousarray(
        np.concatenate([W_qkv[:, 128:256], W_qkv[:, 384:512]], 1)
    ).astype(bf).reshape(32, 128, 256)
    wo = np.ascontiguousarray(W_out).astype(bf)
    bias = np.ascontiguousarray(b_out.reshape(32, 128).T).astype(np.float32)
    return wqa, wqc, wo, bias


def _prep_x(x_core):
    # [64, 256, 256] -> Tt[c, n] with c = c0*64+r1*8+r2, n = hh*32+ww
    t = x_core.reshape(64, 32, 8, 32, 8).transpose(0, 2, 4, 1, 3)
    return np.ascontiguousarray(t.reshape(4096, 1024)).astype(
        ml_dtypes.bfloat16).reshape(32, 128, 1024)


def kernel(x, W_qkv, W_out, b_out):
    x = np.asarray(x, dtype=np.float32)
    W_qkv = np.asarray(W_qkv, dtype=np.float32)
    W_out = np.asarray(W_out, dtype=np.float32)
    b_out = np.asarray(b_out, dtype=np.float32)
    zero_bias = not np.any(b_out)

    debug = bool(int(os.environ.get("KDEBUG", "0")))
    nc = _get_nc(zero_bias, debug_outs=debug)
    wqa, wqc, wo, bias = _prep_weights(W_qkv, W_out, b_out)

    in_maps = [
        {"Tt": _prep_x(x[b]), "WqA": wqa, "WqC": wqc, "Wo": wo, "bias": bias}
        for b in range(8)
    ]
    trace = bool(int(os.environ.get("BENCH_TRACE", "0")))
    if trace:
        try:
            from antenv.axon_hooks import get_axon_ntff_profile_hook  # noqa: F401
        except ImportError:
            trace = False
    res = run_bass_kernel_spmd(nc, in_maps, core_ids=list(range(8)), trace=trace)
    if trace:
        _CACHE["last_result"] = res

    outs = []
    for b in range(8):
        y = np.asarray(res.results[b]["out"]).reshape(4096, 1024)
        y = y.reshape(64, 8, 8, 32, 32)          # [c0, r1, r2, hh, ww]
        y = y.transpose(0, 3, 1, 4, 2)           # [c0, hh, r1, ww, r2]
        outs.append(np.ascontiguousarray(y.reshape(256 * 64, 256)).astype(
            np.float32).reshape(64, 256, 256))
    if debug:
        _CACHE["dbg"] = res.results
    return np.stack(outs)


# revision 3
# speedup vs baseline: 1.0480x; 1.0480x over previous
"""Trainium2 Bass kernel v6 for PixelUnshuffle->MHA->PixelShuffle.

v2 -> v3 (v2 = 239us):
  - ONE PSUM pool for phases B/C/D (pool boundaries in v2 serialized the
    phases behind the softmax-normalization chain).
  - Softmax 1/Z via ACT Ln -> K=1 f32 broadcast matmul -> ACT Exp(scale=-1)
    (v2's InstReciprocal on a single partition cost 6.5us per head and
    stalled the in-order PE queue at every head-pair seam).
  - PE-queue-aware emission order: qk(heads 2,3) groups are emitted BEFORE
    the hp0 norm chain, and the first out-projection ic0 matmuls BEFORE the
    hp1 norm chain, so the PE always has independent work queued ahead of
    any matmul that waits on a slow cross-engine chain.
  - WqC/Wo input DMAs deferred out of phase A's DMA window (A is DMA-bound).
  - Finer first-chunk DMAs + memset/warmup as the very first instructions.

Phases:
  A: qk(heads 0,1) + v(m 0..3)     [own 8-bank pool, DMA-paced]
  B: attention heads 0,1; fillers: v(m 4..7), then qk(heads 2,3) j=0..2
  C: attention heads 2,3; fillers: qk j=3, then out-proj ic0 of cb 0..2
  D: output projection, evac alternating Scalar/Vector, batched out-DMA

Layouts: see docstring of kernel_v2 (unchanged).
"""

import sys

if "/opt/trn_rl_repo" not in sys.path:
    sys.path.insert(0, "/opt/trn_rl_repo")

import os

import ml_dtypes
import numpy as np

import concourse.bass as bass
from concourse import bacc, mybir, tile
from concourse.bass_utils import run_bass_kernel_spmd

F32 = mybir.dt.float32
BF16 = mybir.dt.bfloat16
EXP = mybir.ActivationFunctionType.Exp
LN = mybir.ActivationFunctionType.Ln

SCALE = 0.125  # DIM_HEAD ** -0.5

_CACHE = {}


def _build(zero_bias=True, debug_outs=False):
    nc = bacc.Bacc("TRN2", target_bir_lowering=False, debug=False, num_devices=8)

    tt_d = nc.dram_tensor("Tt", [32, 128, 1024], BF16, kind="ExternalInput").ap()
    wqa_d = nc.dram_tensor("WqA", [32, 128, 512], BF16, kind="ExternalInput").ap()
    wqc_d = nc.dram_tensor("WqC", [32, 128, 256], BF16, kind="ExternalInput").ap()
    wo_d = nc.dram_tensor("Wo", [256, 4096], BF16, kind="ExternalInput").ap()
    b_d = nc.dram_tensor("bias", [128, 32], F32, kind="ExternalInput").ap()
    out_d = nc.dram_tensor("out", [32, 128, 1024], BF16, kind="ExternalOutput").ap()

    dbg = None
    if debug_outs:
        dbg = {
            "qkT": nc.dram_tensor("dbg_qkT", [128, 4, 1024], BF16, kind="ExternalOutput").ap(),
            "v_sb": nc.dram_tensor("dbg_v", [128, 8, 4, 68], BF16, kind="ExternalOutput").ap(),
            "outT": nc.dram_tensor("dbg_outT", [128, 2, 1024], BF16, kind="ExternalOutput").ap(),
        }

    def dram_ap(base, off, pattern):
        return bass.AP(tensor=base.tensor, offset=base.offset + off, ap=pattern)

    with tile.TileContext(nc) as tc:
        _build_tiled(nc, tc, tt_d, wqa_d, wqc_d, wo_d, b_d, out_d, dram_ap,
                     zero_bias, dbg)
    nc.compile()
    return nc


def _build_tiled(nc, tc, tt_d, wqa_d, wqc_d, wo_d, b_d, out_d, dram_ap,
                 zero_bias, dbg=None):
    from contextlib import ExitStack

    def copy_sc(dst, src):
        nc.scalar.copy(dst, src)

    def copy_ve(dst, src):
        nc.vector.tensor_copy(dst, src)

    with ExitStack() as ctx:
        pers = ctx.enter_context(tc.tile_pool(name="pers", bufs=1))

        # ---- persistent SBUF tiles ----
        tt_sb = pers.tile([128, 32, 1024], BF16)   # 64KB/part
        wqa_sb = pers.tile([128, 32, 512], BF16)   # 32KB
        wqc_sb = pers.tile([128, 32, 256], BF16)   # 16KB
        wo_sb = pers.tile([128, 2, 4096], BF16)    # 16KB
        qkT = pers.tile([128, 4, 1024], BF16)      # 8KB
        v_sb = pers.tile([128, 8, 4, 68], BF16)    # 4.25KB
        ed = pers.tile([128, 2, 8, 1024], BF16)    # 32KB (h2, mc, n)
        outT = pers.tile([128, 2, 1024], BF16)     # 4KB
        onrm = pers.tile([64, 1024], BF16)         # h2=1 staging (shared)
        zraw = pers.tile([1, 1024], F32)           # Z row staged to SBUF
        zrec = pers.tile([1, 1024], F32)           # 1/Z, reused per head
        zbc_sb = pers.tile([64, 1024], BF16)       # 1/Z broadcast, reused
        ones64f = pers.tile([1, 64], F32)
        bias_sb = pers.tile([128, 32], F32)
        warm = pers.tile([128, 512], BF16)
        expd = pers.tile([1, 16], BF16)

        # ---- init: DVE memset first so PE warmup can start immediately ----
        nc.vector.memset(warm[:], 0.0)
        nc.vector.memset(ones64f[:], 1.0)
        nc.vector.memset(v_sb[:, :, :, 64:68], 1.0)
        # preload the ln/exp table set (~2.7us) long before it's needed
        nc.scalar.activation(expd[:], warm[0:1, 0:16], EXP)

        # ---- phase-A input DMAs (A is DMA-bound) ----
        # 4-queue rotation; chunk-size ramp: singles first (fast arrival),
        # then pairs, then quads (amortize per-DMA DGE overhead)
        qrot = [nc.gpsimd, nc.scalar, nc.sync, nc.gpsimd]
        qi = 0

        def in_dma(dst, src_off, base, nchunk, width):
            nonlocal_qi = qrot[in_dma.qi % 4]
            in_dma.qi += 1
            nonlocal_qi.dma_start(
                out=dst,
                in_=dram_ap(base, src_off,
                            [[width, 128], [width * 128, nchunk], [1, width]]
                            if nchunk > 1 else [[width, 128], [1, width]]),
            )
        in_dma.qi = 0

        ramp = ([(ci, 1) for ci in range(4)]
                + [(ci, 2) for ci in range(4, 32, 2)])
        for ci, n in ramp:
            in_dma(tt_sb[:, ci : ci + n, :], ci * 131072, tt_d, n, 1024)
            in_dma(wqa_sb[:, ci : ci + n, :], ci * 65536, wqa_d, n, 512)
        # later-phase weights at the tail of the same queues: they issue
        # after every phase-A chunk and stream during early B
        for j in range(4):
            in_dma(wqc_sb[:, 8 * j : 8 * j + 8, :], j * 8 * 32768, wqc_d,
                   8, 256)
        for j in range(2):
            qrot[j].dma_start(
                out=wo_sb[:, :, j * 2048 : (j + 1) * 2048],
                in_=dram_ap(wo_d, j * 2048,
                            [[4096, 128], [524288, 2], [1, 2048]]),
            )
        nc.scalar.dma_start(out=bias_sb[:], in_=dram_ap(b_d, 0, [[32, 128], [1, 32]]))

        # ================= phase A: qk(hp0) + v(m 0..3) =================
        with tc.tile_pool(name="psA", bufs=1, space="PSUM") as psA:
            qk_ps = [
                psA.tile([128, 512], F32, tag="qk", bufs=4, name=f"qkps_{j}")
                for j in range(4)  # j = (ot0,w0) (ot0,w1) (ot2,w0) (ot2,w1)
            ]
            v_ps = [
                psA.tile([128, 256], F32, tag="vv", bufs=4, name=f"vps_{mc}")
                for mc in range(4)
            ]
            # warmup into qk_ps[0]'s bank (real start=True clears it)
            for _ in range(10):
                nc.tensor.matmul(qk_ps[0][:], warm[:, 0:128], warm[:],
                                 start=True, stop=True)
            for ci in range(32):
                first, last = ci == 0, ci == 31
                for j in range(4):
                    ot, w = divmod(j, 2)
                    nc.tensor.matmul(
                        qk_ps[j][:],
                        wqa_sb[:, ci, ot * 128 : (ot + 1) * 128],
                        tt_sb[:, ci, w * 512 : (w + 1) * 512],
                        start=first, stop=last,
                    )
                    nc.tensor.matmul(
                        v_ps[j][:],
                        tt_sb[:, ci, j * 128 : (j + 1) * 128],
                        wqa_sb[:, ci, 256:512],
                        start=first, stop=last,
                    )
            for j in range(4):
                ot, w = divmod(j, 2)
                cp = copy_sc if j % 2 == 0 else copy_ve
                cp(qkT[:, 2 * ot, w * 512 : (w + 1) * 512], qk_ps[j][:])
            for mc in range(4):
                copy_ve(v_sb[:, mc, :, 0:64],
                        v_ps[mc][:].rearrange("p (h d) -> p h d", h=4))

        # ============ phases B, C, D share ONE pool (no barriers) ============
        # tags: rot  3x [128,512] f32  (1 bank each)   - dts / zb / y ring
        #       vrot 1x [128,512]      (1 bank)        - v / qk / zb / y ring
        #       oaug 2x [68,2,512]     (2 banks each)  - oaug pairs / y ring
        with tc.tile_pool(name="psBCD", bufs=1, space="PSUM") as ps:
            def dots_mc(hp, mc):
                for nh in range(2):
                    pair = []
                    for h2 in range(2):
                        dt = ps.tile([128, 512], F32, tag="rot", bufs=3,
                                     name=f"dt{hp}_{mc}_{nh}_{h2}")
                        b = h2 * 64
                        nc.tensor.matmul(
                            dt[:],
                            qkT[b : b + 64, 2 + hp, mc * 128 : (mc + 1) * 128],
                            qkT[b : b + 64, hp, nh * 512 : (nh + 1) * 512],
                            start=True, stop=True,
                        )
                        pair.append(dt)
                    for h2 in range(2):
                        nc.scalar.activation(
                            ed[:, h2, mc, nh * 512 : (nh + 1) * 512],
                            pair[h2][:], EXP, scale=SCALE,
                        )

            def oaug_mc(oaug, hp, mc):
                for h2 in range(2):
                    for nh in range(2):
                        nc.tensor.matmul(
                            oaug[h2][0:68, nh, :],
                            v_sb[:, mc, 2 * hp + h2, :],
                            ed[:, h2, mc, nh * 512 : (nh + 1) * 512],
                            start=mc == 0, stop=mc == 7,
                        )

            def norm_hp(oaug, hp, ztag, zbufs):
                """outT[:, hp] = oaug * (1/Z). DVE fast-reciprocal on the
                PSUM Z row; a K=1 f32 matmul broadcasts 1/Z to 64 partitions
                (PE work that fills the seam); ACT stages it to SBUF."""
                for h2 in range(2):
                    h = 2 * hp + h2
                    nc.scalar.copy(
                        zraw[0:1, :],
                        oaug[h2][64:65, :, :].rearrange("p a b -> p (a b)"),
                    )
                    nc.vector.reciprocal_approx_fast(
                        zrec[0:1, :], zraw[0:1, :])
                    for nh in range(2):
                        zb = ps.tile([64, 512], F32, tag=ztag, bufs=zbufs,
                                     name=f"zb{hp}_{h2}_{nh}")
                        nc.tensor.matmul(
                            zb[:], ones64f[0:1, :],
                            zrec[0:1, nh * 512 : (nh + 1) * 512],
                            start=True, stop=True,
                        )
                        nc.scalar.copy(
                            zbc_sb[:, nh * 512 : (nh + 1) * 512], zb[:],
                        )
                        dst = (outT[0:64, hp, nh * 512 : (nh + 1) * 512]
                               if h2 == 0 else
                               onrm[0:64, nh * 512 : (nh + 1) * 512])
                        nc.vector.tensor_mul(
                            dst,
                            oaug[h2][0:64, nh, :],
                            zbc_sb[:, nh * 512 : (nh + 1) * 512],
                        )
                nc.sync.dma_start(out=outT[64:128, hp, :], in_=onrm[:, :])

            # ---------------- phase B ----------------
            oaugB = [
                ps.tile([68, 2, 512], F32, tag="oaug", bufs=2, name=f"oaugB{h2}")
                for h2 in range(2)
            ]
            dots_mc(0, 0)
            dots_mc(0, 1)
            vg = None
            for mc in range(8):
                g = 4 + mc // 2
                if mc % 2 == 0:
                    vg = ps.tile([128, 256], F32, tag="vrot", bufs=1,
                                 name=f"vpsB_{g}")
                lo, hi = (0, 16) if mc % 2 == 0 else (16, 32)
                for ci in range(lo, hi):
                    nc.tensor.matmul(
                        vg[:],
                        tt_sb[:, ci, g * 128 : (g + 1) * 128],
                        wqa_sb[:, ci, 256:512],
                        start=ci == 0, stop=ci == 31,
                    )
                if mc % 2 == 1:
                    copy_ve(v_sb[:, g, :, 0:64],
                            vg[:].rearrange("p (h d) -> p h d", h=4))
                oaug_mc(oaugB, 0, mc)
                if mc < 6:
                    dots_mc(0, mc + 2)

            # B tail: qk(hp1) groups j=0..2 BEFORE the hp0 norm chain, so the
            # PE has ~10us of queued work while Ln/Exp/muls run cross-engine
            def qk_group_c(j, lo, hi, qp):
                for ci in range(lo, hi):
                    nc.tensor.matmul(
                        qp[:],
                        wqc_sb[:, ci, (j // 2) * 128 : (j // 2 + 1) * 128],
                        tt_sb[:, ci, (j % 2) * 512 : (j % 2 + 1) * 512],
                        start=ci == 0, stop=ci == 31,
                    )

            for j in range(2):
                qp = ps.tile([128, 512], F32, tag="vrot", bufs=1, name=f"qkC{j}")
                qk_group_c(j, 0, 32, qp)
                ot, w = 1 + 2 * (j // 2), j % 2
                copy_ve(qkT[:, ot, w * 512 : (w + 1) * 512], qp[:])
            norm_hp(oaugB, 0, "rot", 3)

            # ---------------- phase C ----------------
            oaugC = [
                ps.tile([68, 2, 512], F32, tag="oaug", bufs=2, name=f"oaugC{h2}")
                for h2 in range(2)
            ]
            # k(ot3) w0 full group first: dots(0..3) read it
            qp2 = ps.tile([128, 512], F32, tag="vrot", bufs=1, name="qkC2")
            qk_group_c(2, 0, 32, qp2)
            copy_ve(qkT[:, 3, 0:512], qp2[:])
            dots_mc(1, 0)
            dots_mc(1, 1)
            qp3 = None
            j3_bursts = [(0, 11), (11, 22), (22, 32)]
            for mc in range(8):
                if mc < 3:  # k(ot3) w1 filler; its evac MUST be emitted
                    # before dots(4)'s emission (same-iteration, later) or
                    # the read binds to no writer
                    if mc == 0:
                        qp3 = ps.tile([128, 512], F32, tag="vrot", bufs=1,
                                      name="qkC3")
                    lo, hi = j3_bursts[mc]
                    qk_group_c(3, lo, hi, qp3)
                    if mc == 2:
                        copy_ve(qkT[:, 3, 512:1024], qp3[:])
                oaug_mc(oaugC, 1, mc)
                if mc < 6:
                    dots_mc(1, mc + 2)

            # seam fill: ic0 of out-projection for cb 0..2 only needs
            # outT[:, 0] (done since B) - queue it ahead of the hp1 norm
            def ymm(yp, cb, half, ic, start, stop):
                nc.tensor.matmul(
                    yp[:],
                    wo_sb[:, ic, cb * 128 : (cb + 1) * 128],
                    outT[:, ic, half * 512 : (half + 1) * 512],
                    start=start, stop=stop,
                )

            # exactly 3 rot slots exist: prefill exactly 3 open ic0
            # accumulations (a 4th would block the PE queue ahead of the
            # norm chain that frees the slots -> deadlock)
            pending = {}
            for cb, half in [(0, 0), (0, 1), (1, 0)]:
                yp = ps.tile([128, 512], F32, tag="rot", bufs=3,
                             name=f"y{cb}_{half}")
                ymm(yp, cb, half, 0, True, False)
                pending[(cb, half)] = yp

            norm_hp(oaugC, 1, "vrot", 1)

            if dbg is not None:
                nc.gpsimd.dma_start(out=dbg["qkT"][:], in_=qkT[:])
                nc.gpsimd.dma_start(out=dbg["v_sb"][:], in_=v_sb[:])
                nc.gpsimd.dma_start(out=dbg["outT"][:], in_=outT[:])

            # ---------------- phase D ----------------
            def yevac(yp, ybf, cb, half):
                dst = ybf[:, half * 512 : (half + 1) * 512]
                if zero_bias:
                    cp = copy_sc if (cb + half) % 2 == 0 else copy_ve
                    cp(dst, yp[:])
                else:
                    nc.vector.tensor_scalar_add(dst, yp[:], bias_sb[:, cb : cb + 1])

            def yout(cb, ybf):
                if cb >= 28:  # split the tail blocks across both queues
                    for hh in range(2):
                        eng = nc.sync if hh == 0 else nc.gpsimd
                        eng.dma_start(
                            out=dram_ap(out_d, cb * 131072 + hh * 512,
                                        [[1024, 128], [1, 512]]),
                            in_=ybf[:, hh * 512 : (hh + 1) * 512],
                        )
                    return
                eng = (nc.sync, nc.gpsimd, nc.scalar)[cb % 3]
                eng.dma_start(
                    out=dram_ap(out_d, cb * 131072, [[1024, 128], [1, 1024]]),
                    in_=ybf[:],
                )

            # remaining blocks: y tiles round-robin across all three tags
            tags = [("rot", 3), ("rot", 3), ("rot", 3), ("oaug", 2),
                    ("oaug", 2)]
            for cb in range(32):
                ybf = pers.tile([128, 1024], BF16, tag="ybf", bufs=5,
                                name=f"ybf{cb}")
                for half in range(2):
                    yp = pending.pop((cb, half), None)
                    if yp is None:
                        tg, tb = tags[(2 * cb + half) % 5]
                        yp = ps.tile([128, 512], F32, tag=tg, bufs=tb,
                                     name=f"y{cb}_{half}")
                        ymm(yp, cb, half, 0, True, False)
                    ymm(yp, cb, half, 1, False, True)
                    yevac(yp, ybf, cb, half)
                yout(cb, ybf)


def _get_nc(zero_bias, debug_outs=False):
    key = f"nc_v6_zb{int(zero_bias)}_dbg{int(debug_outs)}"
    if key not in _CACHE:
        _CACHE[key] = _build(zero_bias=zero_bias, debug_outs=debug_outs)
    return _CACHE[key]


def _prep_weights(W_qkv, W_out, b_out):
    bf = ml_dtypes.bfloat16
    wqa = np.ascontiguousarray(
        np.concatenate([W_qkv[:, 0:128], W_qkv[:, 256:384], W_qkv[:, 512:768]], 1)
    ).astype(bf).reshape(32, 128, 512)
    wqc = np.ascontiguousarray(
        np.concatenate([W_qkv[:, 128:256], W_qkv[:, 384:512]], 1)
    ).astype(bf).reshape(32, 128, 256)
    wo = np.ascontiguousarray(W_out).astype(bf)
    bias = np.ascontiguousarray(b_out.reshape(32, 128).T).astype(np.float32)
    return wqa, wqc, wo, bias


def _prep_x(x_core):
    # [64, 256, 256] -> Tt[c, n] with c = c0*64+r1*8+r2, n = hh*32+ww
    t = x_core.reshape(64, 32, 8, 32, 8).transpose(0, 2, 4, 1, 3)
    return np.ascontiguousarray(t.reshape(4096, 1024)).astype(
        ml_dtypes.bfloat16).reshape(32, 128, 1024)


def kernel(x, W_qkv, W_out, b_out):
    x = np.asarray(x, dtype=np.float32)
    W_qkv = np.asarray(W_qkv, dtype=np.float32)
    W_out = np.asarray(W_out, dtype=np.float32)
    b_out = np.asarray(b_out, dtype=np.float32)
    zero_bias = not np.any(b_out)

    debug = bool(int(os.environ.get("KDEBUG", "0")))
    nc = _get_nc(zero_bias, debug_outs=debug)
    wqa, wqc, wo, bias = _prep_weights(W_qkv, W_out, b_out)

    in_maps = [
        {"Tt": _prep_x(x[b]), "WqA": wqa, "WqC": wqc, "Wo": wo, "bias": bias}
        for b in range(8)
    ]
    trace = bool(int(os.environ.get("BENCH_TRACE", "0")))
    if trace:
        try:
            from antenv.axon_hooks import get_axon_ntff_profile_hook  # noqa: F401
        except ImportError:
            trace = False
    res = run_bass_kernel_spmd(nc, in_maps, core_ids=list(range(8)), trace=trace)
    if trace:
        _CACHE["last_result"] = res

    outs = []
    for b in range(8):
        y = np.asarray(res.results[b]["out"]).reshape(4096, 1024)
        y = y.reshape(64, 8, 8, 32, 32)          # [c0, r1, r2, hh, ww]
        y = y.transpose(0, 3, 1, 4, 2)           # [c0, hh, r1, ww, r2]
        outs.append(np.ascontiguousarray(y.reshape(256 * 64, 256)).astype(
            np.float32).reshape(64, 256, 256))
    if debug:
        _CACHE["dbg"] = res.results
    return np.stack(outs)


# revision 4
# speedup vs baseline: 1.0579x; 1.0094x over previous
"""Trainium2 Bass kernel v9 for PixelUnshuffle->MHA->PixelShuffle.

v2 -> v3 (v2 = 239us):
  - ONE PSUM pool for phases B/C/D (pool boundaries in v2 serialized the
    phases behind the softmax-normalization chain).
  - Softmax 1/Z via ACT Ln -> K=1 f32 broadcast matmul -> ACT Exp(scale=-1)
    (v2's InstReciprocal on a single partition cost 6.5us per head and
    stalled the in-order PE queue at every head-pair seam).
  - PE-queue-aware emission order: qk(heads 2,3) groups are emitted BEFORE
    the hp0 norm chain, and the first out-projection ic0 matmuls BEFORE the
    hp1 norm chain, so the PE always has independent work queued ahead of
    any matmul that waits on a slow cross-engine chain.
  - WqC/Wo input DMAs deferred out of phase A's DMA window (A is DMA-bound).
  - Finer first-chunk DMAs + memset/warmup as the very first instructions.

Phases:
  A: qk(heads 0,1) + v(m 0..3)     [own 8-bank pool, DMA-paced]
  B: attention heads 0,1; fillers: v(m 4..7), then qk(heads 2,3) j=0..2
  C: attention heads 2,3; fillers: qk j=3, then out-proj ic0 of cb 0..2
  D: output projection, evac alternating Scalar/Vector, batched out-DMA

Layouts: see docstring of kernel_v2 (unchanged).
"""

import sys

if "/opt/trn_rl_repo" not in sys.path:
    sys.path.insert(0, "/opt/trn_rl_repo")

import os

import ml_dtypes
import numpy as np

import concourse.bass as bass
from concourse import bacc, mybir, tile
from concourse.bass_utils import run_bass_kernel_spmd

F32 = mybir.dt.float32
BF16 = mybir.dt.bfloat16
EXP = mybir.ActivationFunctionType.Exp
LN = mybir.ActivationFunctionType.Ln

SCALE = 0.125  # DIM_HEAD ** -0.5

_CACHE = {}


def _build(zero_bias=True, debug_outs=False):
    nc = bacc.Bacc("TRN2", target_bir_lowering=False, debug=False, num_devices=8)

    tt_d = nc.dram_tensor("Tt", [128, 32, 1024], BF16, kind="ExternalInput").ap()
    wqa_d = nc.dram_tensor("WqA", [128, 32, 512], BF16, kind="ExternalInput").ap()
    wqc_d = nc.dram_tensor("WqC", [128, 32, 256], BF16, kind="ExternalInput").ap()
    wo_d = nc.dram_tensor("Wo", [256, 4096], BF16, kind="ExternalInput").ap()
    b_d = nc.dram_tensor("bias", [128, 32], F32, kind="ExternalInput").ap()
    out_d = nc.dram_tensor("out", [32, 128, 1024], BF16, kind="ExternalOutput").ap()

    dbg = None
    if debug_outs:
        dbg = {
            "qkT": nc.dram_tensor("dbg_qkT", [128, 4, 1024], BF16, kind="ExternalOutput").ap(),
            "v_sb": nc.dram_tensor("dbg_v", [128, 8, 4, 68], BF16, kind="ExternalOutput").ap(),
            "outT": nc.dram_tensor("dbg_outT", [128, 2, 1024], BF16, kind="ExternalOutput").ap(),
        }

    def dram_ap(base, off, pattern):
        return bass.AP(tensor=base.tensor, offset=base.offset + off, ap=pattern)

    with tile.TileContext(nc) as tc:
        _build_tiled(nc, tc, tt_d, wqa_d, wqc_d, wo_d, b_d, out_d, dram_ap,
                     zero_bias, dbg)
    nc.compile()
    return nc


def _build_tiled(nc, tc, tt_d, wqa_d, wqc_d, wo_d, b_d, out_d, dram_ap,
                 zero_bias, dbg=None):
    from contextlib import ExitStack

    def copy_sc(dst, src):
        nc.scalar.copy(dst, src)

    def copy_ve(dst, src):
        nc.vector.tensor_copy(dst, src)

    with ExitStack() as ctx:
        pers = ctx.enter_context(tc.tile_pool(name="pers", bufs=1))

        # ---- persistent SBUF tiles ----
        tt_sb = pers.tile([128, 32, 1024], BF16)   # 64KB/part
        wqa_sb = pers.tile([128, 32, 512], BF16)   # 32KB
        wqc_sb = pers.tile([128, 32, 256], BF16)   # 16KB
        wo_sb = pers.tile([128, 2, 4096], BF16)    # 16KB
        qkT = pers.tile([128, 4, 1024], BF16)      # 8KB
        v_sb = pers.tile([128, 8, 4, 68], BF16)    # 4.25KB
        ed = pers.tile([128, 2, 8, 1024], BF16)    # 32KB (h2, mc, n)
        outT = pers.tile([128, 2, 1024], BF16)     # 4KB
        onrm = pers.tile([64, 1024], BF16)         # h2=1 staging (shared)
        zraw = pers.tile([1, 1024], F32)           # Z row staged to SBUF
        zrec = pers.tile([1, 1024], F32)           # 1/Z, reused per head
        zbc_sb = pers.tile([64, 1024], BF16)       # 1/Z broadcast, reused
        ones64f = pers.tile([1, 64], F32)
        bias_sb = pers.tile([128, 32], F32)
        warm = pers.tile([128, 512], BF16)
        expd = pers.tile([1, 16], BF16)

        # ---- init: DVE memset first so PE warmup can start immediately ----
        nc.vector.memset(warm[:], 0.0)
        nc.vector.memset(ones64f[:], 1.0)
        nc.vector.memset(v_sb[:, :, :, 64:68], 1.0)
        # preload the ln/exp table set (~2.7us) long before it's needed
        nc.scalar.activation(expd[:], warm[0:1, 0:16], EXP)

        # ---- phase-A input DMAs (A is DMA-bound) ----
        # 4-queue rotation; chunk-size ramp: singles first (fast arrival),
        # then pairs, then quads (amortize per-DMA DGE overhead)
        qrot = [nc.gpsimd, nc.scalar, nc.sync, nc.gpsimd]
        qi = 0

        def in_dma(dst, src_off, base, nchunk, width):
            # DRAM is p-major [128, 32, width]: n chunks are contiguous per p
            eng = qrot[in_dma.qi % 4]
            in_dma.qi += 1
            eng.dma_start(
                out=dst,
                in_=dram_ap(base, src_off,
                            [[width * 32, 128], [1, nchunk * width]]),
            )
        in_dma.qi = 0

        ramp = ([(ci, 1) for ci in range(4)]
                + [(ci, 2) for ci in range(4, 32, 2)])
        for ci, n in ramp:
            in_dma(tt_sb[:, ci : ci + n, :], ci * 1024, tt_d, n, 1024)
            in_dma(wqa_sb[:, ci : ci + n, :], ci * 512, wqa_d, n, 512)
        # later-phase weights at the tail of the same queues: they issue
        # after every phase-A chunk and stream during early B
        for j in range(4):
            in_dma(wqc_sb[:, 8 * j : 8 * j + 8, :], j * 8 * 256, wqc_d,
                   8, 256)
        for j in range(2):
            qrot[j].dma_start(
                out=wo_sb[:, :, j * 2048 : (j + 1) * 2048],
                in_=dram_ap(wo_d, j * 2048,
                            [[4096, 128], [524288, 2], [1, 2048]]),
            )
        nc.scalar.dma_start(out=bias_sb[:], in_=dram_ap(b_d, 0, [[32, 128], [1, 32]]))

        # ================= phase A: qk(hp0) + v(m 0..3) =================
        with tc.tile_pool(name="psA", bufs=1, space="PSUM") as psA:
            qk_ps = [
                psA.tile([128, 512], F32, tag="qk", bufs=4, name=f"qkps_{j}")
                for j in range(4)  # j = (ot0,w0) (ot0,w1) (ot2,w0) (ot2,w1)
            ]
            v_ps = [
                psA.tile([128, 256], F32, tag="vv", bufs=4, name=f"vps_{mc}")
                for mc in range(4)
            ]
            # warmup into qk_ps[0]'s bank (real start=True clears it)
            for _ in range(10):
                nc.tensor.matmul(qk_ps[0][:], warm[:, 0:128], warm[:],
                                 start=True, stop=True)
            for ci in range(32):
                first, last = ci == 0, ci == 31
                for j in range(4):
                    ot, w = divmod(j, 2)
                    nc.tensor.matmul(
                        qk_ps[j][:],
                        wqa_sb[:, ci, ot * 128 : (ot + 1) * 128],
                        tt_sb[:, ci, w * 512 : (w + 1) * 512],
                        start=first, stop=last,
                    )
                    nc.tensor.matmul(
                        v_ps[j][:],
                        tt_sb[:, ci, j * 128 : (j + 1) * 128],
                        wqa_sb[:, ci, 256:512],
                        start=first, stop=last,
                    )
            for j in range(4):
                ot, w = divmod(j, 2)
                cp = copy_sc if j % 2 == 0 else copy_ve
                cp(qkT[:, 2 * ot, w * 512 : (w + 1) * 512], qk_ps[j][:])
            for mc in range(4):
                copy_ve(v_sb[:, mc, :, 0:64],
                        v_ps[mc][:].rearrange("p (h d) -> p h d", h=4))

        # ============ phases B, C, D share ONE pool (no barriers) ============
        # tags: rot  3x [128,512] f32  (1 bank each)   - dts / zb / y ring
        #       vrot 1x [128,512]      (1 bank)        - v / qk / zb / y ring
        #       oaug 2x [68,2,512]     (2 banks each)  - oaug pairs / y ring
        with tc.tile_pool(name="psBCD", bufs=1, space="PSUM") as ps:
            def dots_mc(hp, mc):
                dts = [
                    ps.tile([128, 2, 512], F32, tag="rot", bufs=2,
                            name=f"dt{hp}_{mc}_{h2}")
                    for h2 in range(2)
                ]
                for nh in range(2):
                    for h2 in range(2):  # adjacent pair on row groups 0/64
                        b = h2 * 64
                        nc.tensor.matmul(
                            dts[h2][:, nh, :],
                            qkT[b : b + 64, 2 + hp, mc * 128 : (mc + 1) * 128],
                            qkT[b : b + 64, hp, nh * 512 : (nh + 1) * 512],
                            start=True, stop=True,
                        )
                for h2 in range(2):  # one [128,1024] exp per h2: amortizes
                    nc.scalar.activation(  # the 352-cycle ACT overhead
                        ed[:, h2, mc, :],
                        dts[h2][:, :, :].rearrange("p a b -> p (a b)"),
                        EXP, scale=SCALE,
                    )

            def oaug_one(otile, hp, h2, nh, mc):
                nc.tensor.matmul(
                    otile[0:68, :],
                    v_sb[:, mc, 2 * hp + h2, :],
                    ed[:, h2, mc, nh * 512 : (nh + 1) * 512],
                    start=mc == 0, stop=mc == 7,
                )

            def oaug_mc(oaug, hp, mc):
                for h2 in range(2):
                    oaug_one(oaug[h2], hp, h2, 0, mc)

            def norm_hp(oaug, hp, ztag, zbufs):
                """outT[:, hp] = oaug * (1/Z). hp0's chain runs entirely on
                DVE+PE so it never queues ahead of phase C's exps on the
                Scalar engine; hp1's chain (ACT idle by then) uses ACT for
                the copies to shorten the serial path."""
                stage = copy_ve if hp == 0 else copy_sc
                for h2 in range(2):
                    h = 2 * hp + h2
                    stage(
                        zraw[0:1, :],
                        oaug[h2][64:65, :, :].rearrange("p a b -> p (a b)"),
                    )
                    nc.vector.reciprocal_approx_fast(
                        zrec[0:1, :], zraw[0:1, :])
                    for nh in range(2):
                        zb = ps.tile([64, 512], F32, tag=ztag, bufs=zbufs,
                                     name=f"zb{hp}_{h2}_{nh}")
                        nc.tensor.matmul(
                            zb[:], ones64f[0:1, :],
                            zrec[0:1, nh * 512 : (nh + 1) * 512],
                            start=True, stop=True,
                        )
                        stage(zbc_sb[:, nh * 512 : (nh + 1) * 512], zb[:])
                        dst = (outT[0:64, hp, nh * 512 : (nh + 1) * 512]
                               if h2 == 0 else
                               onrm[0:64, nh * 512 : (nh + 1) * 512])
                        nc.vector.tensor_mul(
                            dst,
                            oaug[h2][0:64, nh, :],
                            zbc_sb[:, nh * 512 : (nh + 1) * 512],
                        )
                nc.sync.dma_start(out=outT[64:128, hp, :], in_=onrm[:, :])

            # ---------------- phase B ----------------
            oaugB = [
                ps.tile([68, 512], F32, tag="oaug", bufs=3, name=f"oaugB{h2}")
                for h2 in range(2)
            ]
            dots_mc(0, 0)
            dots_mc(0, 1)
            vg = None
            for mc in range(8):
                g = 4 + mc // 2
                if mc % 2 == 0:
                    vg = ps.tile([128, 256], F32, tag="vrot", bufs=1,
                                 name=f"vpsB_{g}")
                lo, hi = (0, 16) if mc % 2 == 0 else (16, 32)
                for ci in range(lo, hi):
                    nc.tensor.matmul(
                        vg[:],
                        tt_sb[:, ci, g * 128 : (g + 1) * 128],
                        wqa_sb[:, ci, 256:512],
                        start=ci == 0, stop=ci == 31,
                    )
                if mc % 2 == 1:
                    copy_ve(v_sb[:, g, :, 0:64],
                            vg[:].rearrange("p (h d) -> p h d", h=4))
                oaug_mc(oaugB, 0, mc)
                if mc < 6:
                    dots_mc(0, mc + 2)

            # B tail: qk(hp1) groups j=0..2 BEFORE the hp0 norm chain, so the
            # PE has ~10us of queued work while Ln/Exp/muls run cross-engine
            def qk_group_c(j, lo, hi, qp):
                for ci in range(lo, hi):
                    nc.tensor.matmul(
                        qp[:],
                        wqc_sb[:, ci, (j // 2) * 128 : (j // 2 + 1) * 128],
                        tt_sb[:, ci, (j % 2) * 512 : (j % 2 + 1) * 512],
                        start=ci == 0, stop=ci == 31,
                    )

            def qk_filler(j):
                def f():
                    qp = ps.tile([128, 512], F32, tag="vrot", bufs=1,
                                 name=f"qkC{j}")
                    qk_group_c(j, 0, 32, qp)
                    ot, w = 1 + 2 * (j // 2), j % 2
                    copy_ve(qkT[:, ot, w * 512 : (w + 1) * 512], qp[:])
                return f

            attn_tail(oaugB, 0, "rot", 2, copy_ve,
                      [qk_filler(0), qk_filler(1)])

            # ---------------- phase C ----------------
            oaugC = [
                ps.tile([68, 512], F32, tag="oaug", bufs=3, name=f"oaugC{h2}")
                for h2 in range(2)
            ]
            # k(ot3) w0 full group first: dots(0..3) read it
            qp2 = ps.tile([128, 512], F32, tag="vrot", bufs=1, name="qkC2")
            qk_group_c(2, 0, 32, qp2)
            copy_ve(qkT[:, 3, 0:512], qp2[:])
            dots_mc(1, 0)
            dots_mc(1, 1)
            qp3 = None
            j3_bursts = [(0, 11), (11, 22), (22, 32)]
            for mc in range(8):
                if mc < 3:  # k(ot3) w1 filler; its evac MUST be emitted
                    # before dots(4)'s emission (same-iteration, later) or
                    # the read binds to no writer
                    if mc == 0:
                        qp3 = ps.tile([128, 512], F32, tag="vrot", bufs=1,
                                      name="qkC3")
                    lo, hi = j3_bursts[mc]
                    qk_group_c(3, lo, hi, qp3)
                    if mc == 2:
                        copy_ve(qkT[:, 3, 512:1024], qp3[:])
                oaug_mc(oaugC, 1, mc)
                if mc < 6:
                    dots_mc(1, mc + 2)

            # seam fill: ic0 of out-projection for cb 0..2 only needs
            # outT[:, 0] (done since B) - queue it ahead of the hp1 norm
            def ymm(yp, cb, half, ic, start, stop):
                nc.tensor.matmul(
                    yp[:],
                    wo_sb[:, ic, cb * 128 : (cb + 1) * 128],
                    outT[:, ic, half * 512 : (half + 1) * 512],
                    start=start, stop=stop,
                )

            # exactly 3 rot slots exist: prefill exactly 3 open ic0
            # accumulations (a 4th would block the PE queue ahead of the
            # norm chain that frees the slots -> deadlock)
            pending = {}
            for cb, half in [(0, 0), (0, 1)]:
                yp = ps.tile([128, 512], F32, tag="rot", bufs=2,
                             name=f"y{cb}_{half}")
                ymm(yp, cb, half, 0, True, False)
                pending[(cb, half)] = yp

            attn_tail(oaugC, 1, "vrot", 1, copy_sc, [])

            if dbg is not None:
                nc.gpsimd.dma_start(out=dbg["qkT"][:], in_=qkT[:])
                nc.gpsimd.dma_start(out=dbg["v_sb"][:], in_=v_sb[:])
                nc.gpsimd.dma_start(out=dbg["outT"][:], in_=outT[:])

            # ---------------- phase D ----------------
            def yevac(yp, ybf, cb, half):
                dst = ybf[:, half * 512 : (half + 1) * 512]
                if zero_bias:
                    cp = copy_sc if (cb + half) % 2 == 0 else copy_ve
                    cp(dst, yp[:])
                else:
                    nc.vector.tensor_scalar_add(dst, yp[:], bias_sb[:, cb : cb + 1])

            def yout(cb, ybf):
                if cb >= 28:  # split the tail blocks across both queues
                    for hh in range(2):
                        eng = nc.sync if hh == 0 else nc.gpsimd
                        eng.dma_start(
                            out=dram_ap(out_d, cb * 131072 + hh * 512,
                                        [[1024, 128], [1, 512]]),
                            in_=ybf[:, hh * 512 : (hh + 1) * 512],
                        )
                    return
                eng = (nc.sync, nc.gpsimd, nc.scalar)[cb % 3]
                eng.dma_start(
                    out=dram_ap(out_d, cb * 131072, [[1024, 128], [1, 1024]]),
                    in_=ybf[:],
                )

            # remaining blocks: y tiles round-robin across all three tags
            tags = [("rot", 2), ("rot", 2), ("oaug", 3), ("rot", 2),
                    ("oaug", 3)]
            for cb in range(32):
                ybf = pers.tile([128, 1024], BF16, tag="ybf", bufs=5,
                                name=f"ybf{cb}")
                for half in range(2):
                    yp = pending.pop((cb, half), None)
                    if yp is None:
                        tg, tb = tags[(2 * cb + half) % 5]
                        yp = ps.tile([128, 512], F32, tag=tg, bufs=tb,
                                     name=f"y{cb}_{half}")
                        ymm(yp, cb, half, 0, True, False)
                    ymm(yp, cb, half, 1, False, True)
                    yevac(yp, ybf, cb, half)
                yout(cb, ybf)


def _get_nc(zero_bias, debug_outs=False):
    key = f"nc_v14_zb{int(zero_bias)}_dbg{int(debug_outs)}"
    if key not in _CACHE:
        _CACHE[key] = _build(zero_bias=zero_bias, debug_outs=debug_outs)
    return _CACHE[key]


def _prep_weights(W_qkv, W_out, b_out):
    bf = ml_dtypes.bfloat16
    wqa = np.ascontiguousarray(
        np.concatenate([W_qkv[:, 0:128], W_qkv[:, 256:384], W_qkv[:, 512:768]], 1)
        .reshape(32, 128, 512).transpose(1, 0, 2)).astype(bf)
    wqc = np.ascontiguousarray(
        np.concatenate([W_qkv[:, 128:256], W_qkv[:, 384:512]], 1)
        .reshape(32, 128, 256).transpose(1, 0, 2)).astype(bf)
    wo = np.ascontiguousarray(W_out).astype(bf)
    bias = np.ascontiguousarray(b_out.reshape(32, 128).T).astype(np.float32)
    return wqa, wqc, wo, bias


def _prep_x(x_core):
    # [64, 256, 256] -> Tt[c, n] with c = c0*64+r1*8+r2, n = hh*32+ww
    t = x_core.reshape(64, 32, 8, 32, 8).transpose(0, 2, 4, 1, 3)
    t = t.reshape(32, 128, 1024).transpose(1, 0, 2)
    return np.ascontiguousarray(t).astype(ml_dtypes.bfloat16)


def kernel(x, W_qkv, W_out, b_out):
    x = np.asarray(x, dtype=np.float32)
    W_qkv = np.asarray(W_qkv, dtype=np.float32)
    W_out = np.asarray(W_out, dtype=np.float32)
    b_out = np.asarray(b_out, dtype=np.float32)
    zero_bias = not np.any(b_out)

    debug = bool(int(os.environ.get("KDEBUG", "0")))
    nc = _get_nc(zero_bias, debug_outs=debug)
    wqa, wqc, wo, bias = _prep_weights(W_qkv, W_out, b_out)

    in_maps = [
        {"Tt": _prep_x(x[b]), "WqA": wqa, "WqC": wqc, "Wo": wo, "bias": bias}
        for b in range(8)
    ]
    trace = bool(int(os.environ.get("BENCH_TRACE", "0")))
    if trace:
        try:
            from antenv.axon_hooks import get_axon_ntff_profile_hook  # noqa: F401
        except ImportError:
            trace = False
    res = run_bass_kernel_spmd(nc, in_maps, core_ids=list(range(8)), trace=trace)
    if trace:
        _CACHE["last_result"] = res

    outs = []
    for b in range(8):
        y = np.asarray(res.results[b]["out"]).reshape(4096, 1024)
        y = y.reshape(64, 8, 8, 32, 32)          # [c0, r1, r2, hh, ww]
        y = y.transpose(0, 3, 1, 4, 2)           # [c0, hh, r1, ww, r2]
        outs.append(np.ascontiguousarray(y.reshape(256 * 64, 256)).astype(
            np.float32).reshape(64, 256, 256))
    if debug:
        _CACHE["dbg"] = res.results
    return np.stack(outs)


# revision 5
# speedup vs baseline: 1.0621x; 1.0039x over previous
"""Trainium2 Bass kernel v9 for PixelUnshuffle->MHA->PixelShuffle.

v2 -> v3 (v2 = 239us):
  - ONE PSUM pool for phases B/C/D (pool boundaries in v2 serialized the
    phases behind the softmax-normalization chain).
  - Softmax 1/Z via ACT Ln -> K=1 f32 broadcast matmul -> ACT Exp(scale=-1)
    (v2's InstReciprocal on a single partition cost 6.5us per head and
    stalled the in-order PE queue at every head-pair seam).
  - PE-queue-aware emission order: qk(heads 2,3) groups are emitted BEFORE
    the hp0 norm chain, and the first out-projection ic0 matmuls BEFORE the
    hp1 norm chain, so the PE always has independent work queued ahead of
    any matmul that waits on a slow cross-engine chain.
  - WqC/Wo input DMAs deferred out of phase A's DMA window (A is DMA-bound).
  - Finer first-chunk DMAs + memset/warmup as the very first instructions.

Phases:
  A: qk(heads 0,1) + v(m 0..3)     [own 8-bank pool, DMA-paced]
  B: attention heads 0,1; fillers: v(m 4..7), then qk(heads 2,3) j=0..2
  C: attention heads 2,3; fillers: qk j=3, then out-proj ic0 of cb 0..2
  D: output projection, evac alternating Scalar/Vector, batched out-DMA

Layouts: see docstring of kernel_v2 (unchanged).
"""

import sys

if "/opt/trn_rl_repo" not in sys.path:
    sys.path.insert(0, "/opt/trn_rl_repo")

import os

import ml_dtypes
import numpy as np

import concourse.bass as bass
from concourse import bacc, mybir, tile
from concourse.bass_utils import run_bass_kernel_spmd

F32 = mybir.dt.float32
BF16 = mybir.dt.bfloat16
EXP = mybir.ActivationFunctionType.Exp
LN = mybir.ActivationFunctionType.Ln

SCALE = 0.125  # DIM_HEAD ** -0.5

_CACHE = {}


def _build(zero_bias=True, debug_outs=False):
    nc = bacc.Bacc("TRN2", target_bir_lowering=False, debug=False, num_devices=8)

    tt_d = nc.dram_tensor("Tt", [128, 32, 1024], BF16, kind="ExternalInput").ap()
    wqa_d = nc.dram_tensor("WqA", [128, 32, 512], BF16, kind="ExternalInput").ap()
    wqc_d = nc.dram_tensor("WqC", [128, 32, 256], BF16, kind="ExternalInput").ap()
    wo_d = nc.dram_tensor("Wo", [256, 4096], BF16, kind="ExternalInput").ap()
    b_d = nc.dram_tensor("bias", [128, 32], F32, kind="ExternalInput").ap()
    out_d = nc.dram_tensor("out", [32, 128, 1024], BF16, kind="ExternalOutput").ap()

    dbg = None
    if debug_outs:
        dbg = {
            "qkT": nc.dram_tensor("dbg_qkT", [128, 4, 1024], BF16, kind="ExternalOutput").ap(),
            "v_sb": nc.dram_tensor("dbg_v", [128, 8, 4, 68], BF16, kind="ExternalOutput").ap(),
            "outT": nc.dram_tensor("dbg_outT", [128, 2, 1024], BF16, kind="ExternalOutput").ap(),
        }

    def dram_ap(base, off, pattern):
        return bass.AP(tensor=base.tensor, offset=base.offset + off, ap=pattern)

    with tile.TileContext(nc) as tc:
        _build_tiled(nc, tc, tt_d, wqa_d, wqc_d, wo_d, b_d, out_d, dram_ap,
                     zero_bias, dbg)
    nc.compile()
    return nc


def _build_tiled(nc, tc, tt_d, wqa_d, wqc_d, wo_d, b_d, out_d, dram_ap,
                 zero_bias, dbg=None):
    from contextlib import ExitStack

    def copy_sc(dst, src):
        nc.scalar.copy(dst, src)

    def copy_ve(dst, src):
        nc.vector.tensor_copy(dst, src)

    with ExitStack() as ctx:
        pers = ctx.enter_context(tc.tile_pool(name="pers", bufs=1))

        # ---- persistent SBUF tiles ----
        tt_sb = pers.tile([128, 32, 1024], BF16)   # 64KB/part
        wqa_sb = pers.tile([128, 32, 512], BF16)   # 32KB
        wqc_sb = pers.tile([128, 32, 256], BF16)   # 16KB
        wo_sb = pers.tile([128, 2, 4096], BF16)    # 16KB
        qkT = pers.tile([128, 4, 1024], BF16)      # 8KB
        v_sb = pers.tile([128, 8, 4, 68], BF16)    # 4.25KB
        ed = pers.tile([128, 2, 8, 1024], BF16)    # 32KB (h2, mc, n)
        outT = pers.tile([128, 2, 1024], BF16)     # 4KB
        onrm = pers.tile([64, 1024], BF16)         # h2=1 staging (shared)
        zraw = pers.tile([1, 1024], F32)           # Z row staged to SBUF
        zrec = pers.tile([1, 1024], F32)           # 1/Z, reused per head
        zbc_sb = pers.tile([64, 1024], BF16)       # 1/Z broadcast, reused
        ones64f = pers.tile([1, 64], F32)
        bias_sb = pers.tile([128, 32], F32)
        warm = pers.tile([128, 512], BF16)
        expd = pers.tile([1, 16], BF16)

        # ---- init: DVE memset first so PE warmup can start immediately ----
        nc.vector.memset(warm[:], 0.0)
        nc.vector.memset(ones64f[:], 1.0)
        nc.vector.memset(v_sb[:, :, :, 64:68], 1.0)
        # preload the ln/exp table set (~2.7us) long before it's needed
        nc.scalar.activation(expd[:], warm[0:1, 0:16], EXP)

        # ---- phase-A input DMAs (A is DMA-bound) ----
        # 4-queue rotation; chunk-size ramp: singles first (fast arrival),
        # then pairs, then quads (amortize per-DMA DGE overhead)
        qrot = [nc.gpsimd, nc.scalar, nc.sync, nc.gpsimd]
        qi = 0

        def in_dma(dst, src_off, base, nchunk, width):
            # DRAM is p-major [128, 32, width]: n chunks are contiguous per p
            eng = qrot[in_dma.qi % 4]
            in_dma.qi += 1
            eng.dma_start(
                out=dst,
                in_=dram_ap(base, src_off,
                            [[width * 32, 128], [1, nchunk * width]]),
            )
        in_dma.qi = 0

        ramp = ([(ci, 1) for ci in range(4)]
                + [(ci, 2) for ci in range(4, 32, 2)])
        for ci, n in ramp:
            in_dma(tt_sb[:, ci : ci + n, :], ci * 1024, tt_d, n, 1024)
            in_dma(wqa_sb[:, ci : ci + n, :], ci * 512, wqa_d, n, 512)
        # later-phase weights at the tail of the same queues: they issue
        # after every phase-A chunk and stream during early B
        for j in range(4):
            in_dma(wqc_sb[:, 8 * j : 8 * j + 8, :], j * 8 * 256, wqc_d,
                   8, 256)
        for j in range(2):
            qrot[j].dma_start(
                out=wo_sb[:, :, j * 2048 : (j + 1) * 2048],
                in_=dram_ap(wo_d, j * 2048,
                            [[4096, 128], [524288, 2], [1, 2048]]),
            )
        nc.scalar.dma_start(out=bias_sb[:], in_=dram_ap(b_d, 0, [[32, 128], [1, 32]]))

        # ================= phase A: qk(hp0) + v(m 0..3) =================
        with tc.tile_pool(name="psA", bufs=1, space="PSUM") as psA:
            qk_ps = [
                psA.tile([128, 512], F32, tag="qk", bufs=4, name=f"qkps_{j}")
                for j in range(4)  # j = (ot0,w0) (ot0,w1) (ot2,w0) (ot2,w1)
            ]
            v_ps = [
                psA.tile([128, 256], F32, tag="vv", bufs=4, name=f"vps_{mc}")
                for mc in range(4)
            ]
            # warmup into qk_ps[0]'s bank (real start=True clears it)
            for _ in range(10):
                nc.tensor.matmul(qk_ps[0][:], warm[:, 0:128], warm[:],
                                 start=True, stop=True)
            for ci in range(32):
                first, last = ci == 0, ci == 31
                for j in range(4):
                    ot, w = divmod(j, 2)
                    nc.tensor.matmul(
                        qk_ps[j][:],
                        wqa_sb[:, ci, ot * 128 : (ot + 1) * 128],
                        tt_sb[:, ci, w * 512 : (w + 1) * 512],
                        start=first, stop=last,
                    )
                    nc.tensor.matmul(
                        v_ps[j][:],
                        tt_sb[:, ci, j * 128 : (j + 1) * 128],
                        wqa_sb[:, ci, 256:512],
                        start=first, stop=last,
                    )
            for j in range(4):
                ot, w = divmod(j, 2)
                cp = copy_sc if j % 2 == 0 else copy_ve
                cp(qkT[:, 2 * ot, w * 512 : (w + 1) * 512], qk_ps[j][:])
            for mc in range(4):
                copy_ve(v_sb[:, mc, :, 0:64],
                        v_ps[mc][:].rearrange("p (h d) -> p h d", h=4))

        # ============ phases B, C, D share ONE pool (no barriers) ============
        # tags: rot  3x [128,512] f32  (1 bank each)   - dts / zb / y ring
        #       vrot 1x [128,512]      (1 bank)        - v / qk / zb / y ring
        #       oaug 2x [68,2,512]     (2 banks each)  - oaug pairs / y ring
        with tc.tile_pool(name="psBCD", bufs=1, space="PSUM") as ps:
            def dots_mc(hp, mc):
                dts = [
                    ps.tile([128, 2, 512], F32, tag="rot", bufs=2,
                            name=f"dt{hp}_{mc}_{h2}")
                    for h2 in range(2)
                ]
                for nh in range(2):
                    for h2 in range(2):  # adjacent pair on row groups 0/64
                        b = h2 * 64
                        nc.tensor.matmul(
                            dts[h2][:, nh, :],
                            qkT[b : b + 64, 2 + hp, mc * 128 : (mc + 1) * 128],
                            qkT[b : b + 64, hp, nh * 512 : (nh + 1) * 512],
                            start=True, stop=True,
                        )
                for h2 in range(2):  # one [128,1024] exp per h2: amortizes
                    nc.scalar.activation(  # the 352-cycle ACT overhead
                        ed[:, h2, mc, :],
                        dts[h2][:, :, :].rearrange("p a b -> p (a b)"),
                        EXP, scale=SCALE,
                    )

            def oaug_one(otile, hp, h2, nh, mc):
                nc.tensor.matmul(
                    otile[0:68, :],
                    v_sb[:, mc, 2 * hp + h2, :],
                    ed[:, h2, mc, nh * 512 : (nh + 1) * 512],
                    start=mc == 0, stop=mc == 7,
                )

            def oaug_mc(oaug, hp, mc):
                for h2 in range(2):
                    oaug_one(oaug[h2], hp, h2, 0, mc)

            def norm_hp(oaug, hp, ztag, zbufs):
                """outT[:, hp] = oaug * (1/Z). hp0's chain runs entirely on
                DVE+PE so it never queues ahead of phase C's exps on the
                Scalar engine; hp1's chain (ACT idle by then) uses ACT for
                the copies to shorten the serial path."""
                stage = copy_ve if hp == 0 else copy_sc
                for h2 in range(2):
                    h = 2 * hp + h2
                    stage(
                        zraw[0:1, :],
                        oaug[h2][64:65, :, :].rearrange("p a b -> p (a b)"),
                    )
                    nc.vector.reciprocal_approx_fast(
                        zrec[0:1, :], zraw[0:1, :])
                    for nh in range(2):
                        zb = ps.tile([64, 512], F32, tag=ztag, bufs=zbufs,
                                     name=f"zb{hp}_{h2}_{nh}")
                        nc.tensor.matmul(
                            zb[:], ones64f[0:1, :],
                            zrec[0:1, nh * 512 : (nh + 1) * 512],
                            start=True, stop=True,
                        )
                        stage(zbc_sb[:, nh * 512 : (nh + 1) * 512], zb[:])
                        dst = (outT[0:64, hp, nh * 512 : (nh + 1) * 512]
                               if h2 == 0 else
                               onrm[0:64, nh * 512 : (nh + 1) * 512])
                        nc.vector.tensor_mul(
                            dst,
                            oaug[h2][0:64, nh, :],
                            zbc_sb[:, nh * 512 : (nh + 1) * 512],
                        )
                nc.sync.dma_start(out=outT[64:128, hp, :], in_=onrm[:, :])

            # ---------------- phase B ----------------
            oaugB = [
                ps.tile([68, 512], F32, tag="oaug", bufs=3, name=f"oaugB{h2}")
                for h2 in range(2)
            ]
            dots_mc(0, 0)
            dots_mc(0, 1)
            vg = None
            for mc in range(8):
                g = 4 + mc // 2
                if mc % 2 == 0:
                    vg = ps.tile([128, 256], F32, tag="vrot", bufs=1,
                                 name=f"vpsB_{g}")
                lo, hi = (0, 16) if mc % 2 == 0 else (16, 32)
                for ci in range(lo, hi):
                    nc.tensor.matmul(
                        vg[:],
                        tt_sb[:, ci, g * 128 : (g + 1) * 128],
                        wqa_sb[:, ci, 256:512],
                        start=ci == 0, stop=ci == 31,
                    )
                if mc % 2 == 1:
                    copy_ve(v_sb[:, g, :, 0:64],
                            vg[:].rearrange("p (h d) -> p h d", h=4))
                oaug_mc(oaugB, 0, mc)
                if mc < 6:
                    dots_mc(0, mc + 2)

            # B tail: qk(hp1) groups j=0..2 BEFORE the hp0 norm chain, so the
            # PE has ~10us of queued work while Ln/Exp/muls run cross-engine
            def qk_group_c(j, lo, hi, qp):
                for ci in range(lo, hi):
                    nc.tensor.matmul(
                        qp[:],
                        wqc_sb[:, ci, (j // 2) * 128 : (j // 2 + 1) * 128],
                        tt_sb[:, ci, (j % 2) * 512 : (j % 2 + 1) * 512],
                        start=ci == 0, stop=ci == 31,
                    )

            def qk_filler(j):
                def f():
                    qp = ps.tile([128, 512], F32, tag="vrot", bufs=1,
                                 name=f"qkC{j}")
                    qk_group_c(j, 0, 32, qp)
                    ot, w = 1 + 2 * (j // 2), j % 2
                    copy_ve(qkT[:, ot, w * 512 : (w + 1) * 512], qp[:])
                return f

            attn_tail(oaugB, 0, "rot", 2, copy_ve,
                      [qk_filler(0), qk_filler(1)])

            # ---------------- phase C ----------------
            oaugC = [
                ps.tile([68, 512], F32, tag="oaug", bufs=3, name=f"oaugC{h2}")
                for h2 in range(2)
            ]
            # k(ot3) w0 full group first: dots(0..3) read it
            qp2 = ps.tile([128, 512], F32, tag="vrot", bufs=1, name="qkC2")
            qk_group_c(2, 0, 32, qp2)
            copy_ve(qkT[:, 3, 0:512], qp2[:])
            dots_mc(1, 0)
            dots_mc(1, 1)
            qp3 = None
            j3_bursts = [(0, 11), (11, 22), (22, 32)]
            for mc in range(8):
                if mc < 3:  # k(ot3) w1 filler; its evac MUST be emitted
                    # before dots(4)'s emission (same-iteration, later) or
                    # the read binds to no writer
                    if mc == 0:
                        qp3 = ps.tile([128, 512], F32, tag="vrot", bufs=1,
                                      name="qkC3")
                    lo, hi = j3_bursts[mc]
                    qk_group_c(3, lo, hi, qp3)
                    if mc == 2:
                        copy_ve(qkT[:, 3, 512:1024], qp3[:])
                oaug_mc(oaugC, 1, mc)
                if mc < 6:
                    dots_mc(1, mc + 2)

            # seam fill: ic0 of out-projection for cb 0..2 only needs
            # outT[:, 0] (done since B) - queue it ahead of the hp1 norm
            def ymm(yp, cb, half, ic, start, stop):
                nc.tensor.matmul(
                    yp[:],
                    wo_sb[:, ic, cb * 128 : (cb + 1) * 128],
                    outT[:, ic, half * 512 : (half + 1) * 512],
                    start=start, stop=stop,
                )

            # exactly 3 rot slots exist: prefill exactly 3 open ic0
            # accumulations (a 4th would block the PE queue ahead of the
            # norm chain that frees the slots -> deadlock)
            pending = {}
            for cb, half in [(0, 0), (0, 1)]:
                yp = ps.tile([128, 512], F32, tag="rot", bufs=2,
                             name=f"y{cb}_{half}")
                ymm(yp, cb, half, 0, True, False)
                pending[(cb, half)] = yp

            attn_tail(oaugC, 1, "vrot", 1, copy_sc, [])

            if dbg is not None:
                nc.gpsimd.dma_start(out=dbg["qkT"][:], in_=qkT[:])
                nc.gpsimd.dma_start(out=dbg["v_sb"][:], in_=v_sb[:])
                nc.gpsimd.dma_start(out=dbg["outT"][:], in_=outT[:])

            # ---------------- phase D ----------------
            def yevac(yp, ybf, cb, half):
                dst = ybf[:, half * 512 : (half + 1) * 512]
                if zero_bias:
                    cp = copy_sc if (cb + half) % 2 == 0 else copy_ve
                    cp(dst, yp[:])
                else:
                    nc.vector.tensor_scalar_add(dst, yp[:], bias_sb[:, cb : cb + 1])

            def yout(cb, ybf):
                if cb >= 28:  # split the tail blocks across both queues
                    for hh in range(2):
                        eng = nc.sync if hh == 0 else nc.gpsimd
                        eng.dma_start(
                            out=dram_ap(out_d, cb * 131072 + hh * 512,
                                        [[1024, 128], [1, 512]]),
                            in_=ybf[:, hh * 512 : (hh + 1) * 512],
                        )
                    return
                eng = (nc.sync, nc.gpsimd, nc.scalar)[cb % 3]
                eng.dma_start(
                    out=dram_ap(out_d, cb * 131072, [[1024, 128], [1, 1024]]),
                    in_=ybf[:],
                )

            # remaining blocks: y tiles round-robin across all three tags
            tags = [("rot", 2), ("rot", 2), ("oaug", 3), ("rot", 2),
                    ("oaug", 3)]
            for cb in range(32):
                ybf = pers.tile([128, 1024], BF16, tag="ybf", bufs=5,
                                name=f"ybf{cb}")
                for half in range(2):
                    yp = pending.pop((cb, half), None)
                    if yp is None:
                        tg, tb = tags[(2 * cb + half) % 5]
                        yp = ps.tile([128, 512], F32, tag=tg, bufs=tb,
                                     name=f"y{cb}_{half}")
                        ymm(yp, cb, half, 0, True, False)
                    ymm(yp, cb, half, 1, False, True)
                    yevac(yp, ybf, cb, half)
                yout(cb, ybf)


def _get_nc(zero_bias, debug_outs=False):
    key = f"nc_v15_zb{int(zero_bias)}_dbg{int(debug_outs)}"
    if key not in _CACHE:
        _CACHE[key] = _build(zero_bias=zero_bias, debug_outs=debug_outs)
    return _CACHE[key]


def _prep_weights(W_qkv, W_out, b_out):
    bf = ml_dtypes.bfloat16
    wqa = np.ascontiguousarray(
        np.concatenate([W_qkv[:, 0:128], W_qkv[:, 256:384], W_qkv[:, 512:768]], 1)
        .reshape(32, 128, 512).transpose(1, 0, 2)).astype(bf)
    wqc = np.ascontiguousarray(
        np.concatenate([W_qkv[:, 128:256], W_qkv[:, 384:512]], 1)
        .reshape(32, 128, 256).transpose(1, 0, 2)).astype(bf)
    wo = np.ascontiguousarray(W_out).astype(bf)
    bias = np.ascontiguousarray(b_out.reshape(32, 128).T).astype(np.float32)
    return wqa, wqc, wo, bias


def _prep_x(x_core):
    # [64, 256, 256] -> Tt[c, n] with c = c0*64+r1*8+r2, n = hh*32+ww
    t = x_core.reshape(64, 32, 8, 32, 8).transpose(0, 2, 4, 1, 3)
    t = t.reshape(32, 128, 1024).transpose(1, 0, 2)
    return np.ascontiguousarray(t).astype(ml_dtypes.bfloat16)


def kernel(x, W_qkv, W_out, b_out):
    x = np.asarray(x, dtype=np.float32)
    W_qkv = np.asarray(W_qkv, dtype=np.float32)
    W_out = np.asarray(W_out, dtype=np.float32)
    b_out = np.asarray(b_out, dtype=np.float32)
    zero_bias = not np.any(b_out)

    debug = bool(int(os.environ.get("KDEBUG", "0")))
    nc = _get_nc(zero_bias, debug_outs=debug)
    wqa, wqc, wo, bias = _prep_weights(W_qkv, W_out, b_out)

    in_maps = [
        {"Tt": _prep_x(x[b]), "WqA": wqa, "WqC": wqc, "Wo": wo, "bias": bias}
        for b in range(8)
    ]
    trace = bool(int(os.environ.get("BENCH_TRACE", "0")))
    if trace:
        try:
            from antenv.axon_hooks import get_axon_ntff_profile_hook  # noqa: F401
        except ImportError:
            trace = False
    res = run_bass_kernel_spmd(nc, in_maps, core_ids=list(range(8)), trace=trace)
    if trace:
        _CACHE["last_result"] = res

    outs = []
    for b in range(8):
        y = np.asarray(res.results[b]["out"]).reshape(4096, 1024)
        y = y.reshape(64, 8, 8, 32, 32)          # [c0, r1, r2, hh, ww]
        y = y.transpose(0, 3, 1, 4, 2)           # [c0, hh, r1, ww, r2]
        outs.append(np.ascontiguousarray(y.reshape(256 * 64, 256)).astype(
            np.float32).reshape(64, 256, 256))
    if debug:
        _CACHE["dbg"] = res.results
    return np.stack(outs)
